# revision 3
# baseline (speedup 1.0000x reference)
"""Trainium2 Bass kernel for nn_Discriminator (GRU over [64, 1024, 1024]).

Self-contained: builds an SPMD Bass/Tile kernel for 8 NeuronCores,
batch-parallel (8 batch rows per core), runs it via PJRT on the axon
devices, and applies the tiny output head on the host.

Layout ("tile-slot"): SBUF tensors are [128 partitions, (j, b)] where
hidden index hid = j*128 + p (j = k-tile 0..7), b = local batch 0..7.
Phase 1 (x @ W_ih.T) uses a chunk-major x layout and grouped contiguous
xg writes so all large DMAs are contiguous; phase 2 runs the sequential
GRU scan with W_hh SBUF-resident as the stationary matmul operand
(bf16 fast weight load, fp32 PSUM accum), gate order r->n->z so the
long n-gate elementwise chain overlaps the z-gate matmuls and only the
short z chain (z_pre, sigmoid, zd, h') trails each step.
`reps` repeats the whole body inside one NEFF for slope timing.
"""

import numpy as np
import ml_dtypes

import jax
from jax.sharding import Mesh, PartitionSpec, NamedSharding
from jax.experimental.shard_map import shard_map

import concourse.bass as bass
import concourse.mybir as mybir
import concourse.tile as tile
from concourse import bacc, bass2jax
from concourse.bass import ds

F32 = mybir.dt.float32
BF16 = mybir.dt.bfloat16
AF = mybir.ActivationFunctionType
OP = mybir.AluOpType

B, T, S = 64, 1024, 1024
N_CORES = 8
BC = B // N_CORES      # 8 local batch rows
KT = S // 128          # 8 hidden k-tiles
MT = 3 * KT            # 24 gate m-tiles
JB = KT * BC           # 64 slot-layout free size
NCHUNK = 512
NXP = (T * BC) // NCHUNK
TB = 8                 # timesteps per scan block
NBLK = T // TB


def _build():
    nc = bacc.Bacc("TRN2", target_bir_lowering=False, num_devices=N_CORES)

    xT = nc.dram_tensor("xT", [NXP, 128, KT * NCHUNK], BF16, kind="ExternalInput")
    wih = nc.dram_tensor("wih", [S, 3 * S], BF16, kind="ExternalInput")
    whh = nc.dram_tensor("whh", [S, 3 * S], BF16, kind="ExternalInput")
    biasm = nc.dram_tensor("biasm", [128, MT], F32, kind="ExternalInput")
    bhhn = nc.dram_tensor("bhhn", [128, JB], F32, kind="ExternalInput")
    hT_out = nc.dram_tensor("hT", [128, JB], F32, kind="ExternalOutput")

    MG = 4                 # m-tiles per contiguous xg write
    NMG = MT // MG         # 6 write groups
    xg = nc.dram_tensor("xg_scratch", [NXP, NMG, 128, MG * NCHUNK], BF16)
    # phase-1 write view: [p, chunk, group, m-in-group * f]
    xgW = xg.rearrange("c g p f -> p c g f")
    # scan read view: [p, chunk, group, m-in-group, q(=8 blocks), f(=TB*BC)]
    xgS = xg.rearrange("c g p (m q f) -> p c g m q f", m=MG, q=NCHUNK // (TB * BC))

    wih_sb = nc.alloc_sbuf_tensor("wih_sb", [128, KT * 3 * S], BF16)
    whh_sb = nc.alloc_sbuf_tensor("whh_sb", [128, KT * 3 * S], BF16)
    biasm_sb = nc.alloc_sbuf_tensor("biasm_sb", [128, MT], F32)
    bhhn_sb = nc.alloc_sbuf_tensor("bhhn_sb", [128, JB], F32)
    h_a = nc.alloc_sbuf_tensor("h_a", [128, JB], BF16)
    h_b = nc.alloc_sbuf_tensor("h_b", [128, JB], BF16)

    wihR = wih.rearrange("(k p) g -> p k g", p=128)
    whhR = whh.rearrange("(k p) g -> p k g", p=128)
    xTr = xT.rearrange("c p f -> p c f")
    G3S = 3 * S

    with tile.TileContext(nc) as tc:
        nc.sync.dma_start(out=wih_sb[:, :].rearrange("p (k g) -> p k g", k=KT), in_=wihR)
        nc.sync.dma_start(out=whh_sb[:, :].rearrange("p (k g) -> p k g", k=KT), in_=whhR)
        nc.sync.dma_start(out=biasm_sb[:, :], in_=biasm[:, :])
        nc.sync.dma_start(out=bhhn_sb[:, :], in_=bhhn[:, :])
        nc.vector.memset(h_a[:, :], 0.0)

        # phase 1: xg = W_ih @ x.T + bias
        QB = NCHUNK // (TB * BC)
        with tc.tile_pool(name="xp_sb", bufs=3) as xp_pool, \
             tc.tile_pool(name="xp_ps", bufs=4, space="PSUM") as ps_pool, \
             tc.tile_pool(name="xp_ev", bufs=4) as ev_pool:
            with tc.For_i(0, NXP, 1, hint_engines=(mybir.EngineType.PE,)) as c:
                xt_t = xp_pool.tile([128, KT * NCHUNK], BF16)
                nc.sync.dma_start(
                    out=xt_t[:, :], in_=xTr[:, ds(c, 1), :].rearrange("p o f -> p (o f)")
                )
                for g in range(NMG):
                    ev = ev_pool.tile([128, MG * NCHUNK], BF16)
                    for mi in range(MG):
                        m = g * MG + mi
                        ps = ps_pool.tile([128, NCHUNK], F32)
                        for k in range(KT):
                            nc.tensor.matmul(
                                ps[:, :],
                                wih_sb[:, k * G3S + m * 128 : k * G3S + (m + 1) * 128],
                                xt_t[:, k * NCHUNK : (k + 1) * NCHUNK],
                                start=(k == 0),
                                stop=(k == KT - 1),
                            )
                        nc.vector.tensor_scalar_add(
                            ev[:, mi * NCHUNK : (mi + 1) * NCHUNK],
                            ps[:, :],
                            biasm_sb[:, m : m + 1],
                        )
                    nc.sync.dma_start(
                        out=xgW[:, ds(c, 1), g, :].rearrange("p o f -> p (o f)"),
                        in_=ev[:, :],
                    )

        # phase 2: GRU scan
        with tc.tile_pool(name="sc_xg", bufs=2) as xg_pool, \
             tc.tile_pool(name="sc_ps", bufs=4, space="PSUM") as sps_pool, \
             tc.tile_pool(name="sc_ew", bufs=6) as ew_pool:
            with tc.For_i(0, NBLK, 1, hint_engines=(mybir.EngineType.PE,)) as blk:
                xgb = xg_pool.tile([128, MT * TB * BC], BF16)
                QBK = NCHUNK // (TB * BC)  # 1 chunk = 8 scan blocks
                for gg in range(NMG):
                    nc.sync.dma_start(
                        out=xgb[:, gg * MG * TB * BC : (gg + 1) * MG * TB * BC]
                        .rearrange("p (m f) -> p m f", m=MG),
                        in_=xgS[
                            :, ds(blk // QBK, 1), ds(gg, 1), :, ds(blk % QBK, 1), :
                        ].rearrange("p c g m q f -> p (c g m) (q f)"),
                    )
                xgb3 = xgb[:, :].rearrange("p (m f) -> p m f", m=MT)
                for tp in range(TB):
                    h_cur = h_a if tp % 2 == 0 else h_b
                    h_nxt = h_b if tp % 2 == 0 else h_a
                    ps_rz = sps_pool.tile([128, 128], F32, tag="ps_rz")
                    ps_n = sps_pool.tile([128, JB], F32, tag="ps_n")
                    for g in (0, 2, 1):
                        for j in range(KT):
                            m = g * KT + j
                            out_ap = (
                                ps_rz[:, g * 64 + j * BC : g * 64 + (j + 1) * BC]
                                if g < 2
                                else ps_n[:, j * BC : (j + 1) * BC]
                            )
                            for k in range(KT):
                                nc.tensor.matmul(
                                    out_ap,
                                    whh_sb[:, k * G3S + m * 128 : k * G3S + (m + 1) * 128],
                                    h_cur[:, k * BC : (k + 1) * BC],
                                    start=(k == 0),
                                    stop=(k == KT - 1),
                                )
                    xg_r = xgb3[:, 0:8, tp * BC : (tp + 1) * BC]
                    xg_z = xgb3[:, 8:16, tp * BC : (tp + 1) * BC]
                    xg_n = xgb3[:, 16:24, tp * BC : (tp + 1) * BC]
                    # r chain (ready after r matmuls; overlaps n matmuls)
                    r_pre = ew_pool.tile([128, JB], F32, tag="r_pre")
                    nc.vector.tensor_tensor(
                        r_pre[:, :].rearrange("p (m f) -> p m f", m=KT),
                        ps_rz[:, 0:64].rearrange("p (m f) -> p m f", m=KT),
                        xg_r,
                        OP.add,
                    )
                    sig_r = ew_pool.tile([128, JB], BF16, tag="sig_r")
                    nc.scalar.activation(sig_r[:, :], r_pre[:, :], AF.Sigmoid)
                    # n chain (ready after n matmuls; overlaps z matmuls)
                    hn = ew_pool.tile([128, JB], F32, tag="hn")
                    nc.vector.tensor_tensor(hn[:, :], ps_n[:, :], bhhn_sb[:, :], OP.add)
                    rhn = ew_pool.tile([128, JB], F32, tag="rhn")
                    nc.vector.tensor_tensor(rhn[:, :], hn[:, :], sig_r[:, :], OP.mult)
                    n_pre = ew_pool.tile([128, JB], F32, tag="n_pre")
                    nc.vector.tensor_tensor(
                        n_pre[:, :].rearrange("p (m f) -> p m f", m=KT),
                        rhn[:, :].rearrange("p (m f) -> p m f", m=KT),
                        xg_n,
                        OP.add,
                    )
                    n_t = ew_pool.tile([128, JB], BF16, tag="n_t")
                    nc.scalar.activation(n_t[:, :], n_pre[:, :], AF.Tanh)
                    d_t = ew_pool.tile([128, JB], BF16, tag="d_t")
                    nc.vector.tensor_tensor(d_t[:, :], h_cur[:, :], n_t[:, :], OP.subtract)
                    # z chain (the only post-z-matmul critical path)
                    z_pre = ew_pool.tile([128, JB], F32, tag="z_pre")
                    nc.vector.tensor_tensor(
                        z_pre[:, :].rearrange("p (m f) -> p m f", m=KT),
                        ps_rz[:, 64:128].rearrange("p (m f) -> p m f", m=KT),
                        xg_z,
                        OP.add,
                    )
                    sig_z = ew_pool.tile([128, JB], BF16, tag="sig_z")
                    nc.scalar.activation(sig_z[:, :], z_pre[:, :], AF.Sigmoid)
                    zd = ew_pool.tile([128, JB], BF16, tag="zd")
                    nc.vector.tensor_tensor(zd[:, :], d_t[:, :], sig_z[:, :], OP.mult)
                    nc.vector.tensor_tensor(h_nxt[:, :], zd[:, :], n_t[:, :], OP.add)

        hT_sb = nc.alloc_sbuf_tensor("hT_sb", [128, JB], F32)
        nc.vector.tensor_copy(hT_sb[:, :], h_a[:, :])
        nc.sync.dma_start(out=hT_out[:, :], in_=hT_sb[:, :])

    nc.compile()
    return nc


def _prep_inputs(inputs):
    batch = np.asarray(inputs["batch"], np.float32)
    W_ih = np.asarray(inputs["W_ih"], np.float32)
    W_hh = np.asarray(inputs["W_hh"], np.float32)
    b_ih = np.asarray(inputs["b_ih"], np.float32)
    b_hh = np.asarray(inputs["b_hh"], np.float32)

    wihT = np.ascontiguousarray(W_ih.T).astype(ml_dtypes.bfloat16)
    whhT = np.ascontiguousarray(W_hh.T).astype(ml_dtypes.bfloat16)
    bias = b_ih.copy()
    bias[: 2 * S] += b_hh[: 2 * S]
    biasm = np.ascontiguousarray(bias.reshape(MT, 128).T).astype(np.float32)
    bn = b_hh[2 * S :].reshape(KT, 128).T
    bhhn = np.repeat(bn[:, :, None], BC, axis=2).reshape(128, JB).astype(np.float32)

    in_maps = []
    for c in range(N_CORES):
        bs = batch[c * BC : (c + 1) * BC]
        xTc = bs.transpose(2, 1, 0).reshape(S, T * BC)
        # chunk-major layout: [chunk, partition, k-tile * in-chunk-token]
        xTc = np.ascontiguousarray(
            xTc.reshape(KT, 128, NXP, NCHUNK)
            .transpose(2, 1, 0, 3)
            .reshape(NXP, 128, KT * NCHUNK)
        )
        in_maps.append({
            "xT": xTc.astype(ml_dtypes.bfloat16),
            "wih": wihT,
            "whh": whhT,
            "biasm": biasm,
            "bhhn": bhhn,
        })
    return in_maps


class _Runner:
    """Compile once, keep the PJRT executable; run per-core in_maps SPMD."""

    def __init__(self, nc, n_cores):
        bass2jax.install_neuronx_cc_hook()
        self.nc, self.n_cores = nc, n_cores
        pname = nc.partition_id_tensor.name if nc.partition_id_tensor else None
        in_names, out_names, out_avals = [], [], []
        for alloc in nc.m.functions[0].allocations:
            if not isinstance(alloc, mybir.MemoryLocationSet):
                continue
            name = alloc.memorylocations[0].name
            if alloc.kind == "ExternalInput":
                if name != pname:
                    in_names.append(name)
            elif alloc.kind == "ExternalOutput":
                out_names.append(name)
                out_avals.append(
                    jax.core.ShapedArray(tuple(alloc.tensor_shape), mybir.dt.np(alloc.dtype))
                )
        self.in_names, self.out_names, self.out_avals = in_names, out_names, out_avals
        n_params = len(in_names)
        all_in = list(in_names) + list(out_names)
        if pname is not None:
            all_in.append(pname)

        def _body(*args):
            operands = list(args)
            if pname is not None:
                operands.append(bass2jax.partition_id_tensor())
            return tuple(
                bass2jax._bass_exec_p.bind(
                    *operands,
                    out_avals=tuple(out_avals),
                    in_names=tuple(all_in),
                    out_names=tuple(out_names),
                    lowering_input_output_aliases=(),
                    sim_require_finite=False,
                    sim_require_nnan=False,
                    nc=nc,
                )
            )

        devices = jax.devices()[:n_cores]
        self.mesh = Mesh(np.asarray(devices), ("core",))
        specs = (PartitionSpec("core"),) * (n_params + len(out_names))
        self.fn = jax.jit(
            shard_map(_body, mesh=self.mesh, in_specs=specs,
                      out_specs=(PartitionSpec("core"),) * len(out_names),
                      check_rep=False),
            keep_unused=True,
        )

    def concat_inputs(self, in_maps):
        concat = [
            np.concatenate([np.asarray(in_maps[c][n]) for c in range(self.n_cores)], axis=0)
            for n in self.in_names
        ]
        zeros = [
            np.zeros((self.n_cores * a.shape[0], *a.shape[1:]), a.dtype)
            for a in self.out_avals
        ]
        return concat + zeros

    def put(self, arrays):
        sh = NamedSharding(self.mesh, PartitionSpec("core"))
        return [jax.device_put(a, sh) for a in arrays]

    def run(self, in_maps):
        dev = self.put(self.concat_inputs(in_maps))
        outs = self.fn(*dev)
        jax.block_until_ready(outs)
        return [
            {
                n: np.asarray(outs[i]).reshape(self.n_cores, *self.out_avals[i].shape)[c]
                for i, n in enumerate(self.out_names)
            }
            for c in range(self.n_cores)
        ]


_CACHED = {}


def kernel(**inputs) -> np.ndarray:
    if "runner" not in _CACHED:
        _CACHED["nc"] = _build()
        _CACHED["runner"] = _Runner(_CACHED["nc"], N_CORES)
    runner = _CACHED["runner"]
    in_maps = _prep_inputs(inputs)
    results = runner.run(in_maps)

    W_out = np.asarray(inputs["W_out"], np.float32)
    b_out = np.asarray(inputs["b_out"], np.float32)
    outs = []
    for c in range(N_CORES):
        hT = np.asarray(results[c]["hT"], np.float32)
        h = hT.reshape(128, KT, BC).transpose(2, 1, 0).reshape(BC, S)
        logits = h @ W_out.T + b_out
        outs.append(1.0 / (1.0 + np.exp(-logits[:, 0])))
    return np.concatenate(outs, 0).astype(np.float32)



# revision 5
# speedup vs baseline: 1.0466x; 1.0466x over previous
"""Trainium2 Bass kernel for nn_Discriminator (GRU over [64, 1024, 1024]).

Self-contained: builds an SPMD Bass/Tile kernel for 8 NeuronCores,
batch-parallel (8 batch rows per core), runs it via PJRT on the axon
devices, and applies the tiny output head on the host.

Layout ("tile-slot"): SBUF tensors are [128 partitions, (j, b)] where
hidden index hid = j*128 + p (j = k-tile 0..7), b = local batch 0..7.
Phase 1 (x @ W_ih.T) uses a chunk-major x layout and grouped contiguous
xg writes so all large DMAs are contiguous; phase 2 runs the sequential
GRU scan with W_hh SBUF-resident as the stationary matmul operand
(bf16 fast weight load, fp32 PSUM accum), gate order r->n->z so the
long n-gate elementwise chain overlaps the z-gate matmuls and only the
short z chain (z_pre, sigmoid, zd, h') trails each step.
`reps` repeats the whole body inside one NEFF for slope timing.
"""

import numpy as np
import ml_dtypes

import jax
from jax.sharding import Mesh, PartitionSpec, NamedSharding
from jax.experimental.shard_map import shard_map

import concourse.bass as bass
import concourse.mybir as mybir
import concourse.tile as tile
from concourse import bacc, bass2jax
from concourse.bass import ds

F32 = mybir.dt.float32
BF16 = mybir.dt.bfloat16
AF = mybir.ActivationFunctionType
OP = mybir.AluOpType

B, T, S = 64, 1024, 1024
N_CORES = 8
BC = B // N_CORES      # 8 local batch rows
KT = S // 128          # 8 hidden k-tiles
MT = 3 * KT            # 24 gate m-tiles
JB = KT * BC           # 64 slot-layout free size
NCHUNK = 512
NXP = (T * BC) // NCHUNK
TB = 8                 # timesteps per scan block
NBLK = T // TB


def _build():
    nc = bacc.Bacc("TRN2", target_bir_lowering=False, num_devices=N_CORES)

    xT = nc.dram_tensor("xT", [NXP, 128, KT * NCHUNK], BF16, kind="ExternalInput")
    wih = nc.dram_tensor("wih", [S, 3 * S], BF16, kind="ExternalInput")
    whh = nc.dram_tensor("whh", [S, 3 * S], BF16, kind="ExternalInput")
    biasm = nc.dram_tensor("biasm", [128, MT], F32, kind="ExternalInput")
    bhhn = nc.dram_tensor("bhhn", [128, JB], F32, kind="ExternalInput")
    hT_out = nc.dram_tensor("hT", [128, JB], F32, kind="ExternalOutput")

    MG = 4                 # m-tiles per contiguous xg write
    NMG = MT // MG         # 6 write groups
    xg = nc.dram_tensor("xg_scratch", [NXP, NMG, 128, MG * NCHUNK], BF16)
    # phase-1 write view: [p, chunk, group, m-in-group * f]
    xgW = xg.rearrange("c g p f -> p c g f")
    # scan read view: [p, chunk, group, m-in-group, q(blocks/chunk), f(=tb*BC)]
    xgS = xg.rearrange("c g p (m q f) -> p c g m q f", m=MG, q=NCHUNK // (tb * BC))

    wih_sb = nc.alloc_sbuf_tensor("wih_sb", [128, KT * 3 * S], BF16)
    whh_sb = nc.alloc_sbuf_tensor("whh_sb", [128, KT * 3 * S], BF16)
    biasm_sb = nc.alloc_sbuf_tensor("biasm_sb", [128, MT], F32)
    bhhn_sb = nc.alloc_sbuf_tensor("bhhn_sb", [128, JB], F32)
    h_a = nc.alloc_sbuf_tensor("h_a", [128, JB], BF16)
    h_b = nc.alloc_sbuf_tensor("h_b", [128, JB], BF16)

    wihR = wih.rearrange("(k p) g -> p k g", p=128)
    whhR = whh.rearrange("(k p) g -> p k g", p=128)
    xTr = xT.rearrange("c p f -> p c f")
    G3S = 3 * S

    with tile.TileContext(nc) as tc:
        nc.sync.dma_start(out=wih_sb[:, :].rearrange("p (k g) -> p k g", k=KT), in_=wihR)
        nc.sync.dma_start(out=whh_sb[:, :].rearrange("p (k g) -> p k g", k=KT), in_=whhR)
        nc.sync.dma_start(out=biasm_sb[:, :], in_=biasm[:, :])
        nc.sync.dma_start(out=bhhn_sb[:, :], in_=bhhn[:, :])
        nc.vector.memset(h_a[:, :], 0.0)

        # phase 1: xg = W_ih @ x.T + bias
        QB = NCHUNK // (TB * BC)
        with tc.tile_pool(name="xp_sb", bufs=3) as xp_pool, \
             tc.tile_pool(name="xp_ps", bufs=4, space="PSUM") as ps_pool, \
             tc.tile_pool(name="xp_ev", bufs=4) as ev_pool:
            with tc.For_i(0, NXP, 1, hint_engines=(mybir.EngineType.PE,)) as c:
                xt_t = xp_pool.tile([128, KT * NCHUNK], BF16)
                nc.sync.dma_start(
                    out=xt_t[:, :], in_=xTr[:, ds(c, 1), :].rearrange("p o f -> p (o f)")
                )
                for g in range(NMG):
                    ev = ev_pool.tile([128, MG * NCHUNK], BF16)
                    for mi in range(MG):
                        m = g * MG + mi
                        ps = ps_pool.tile([128, NCHUNK], F32)
                        for k in range(KT):
                            nc.tensor.matmul(
                                ps[:, :],
                                wih_sb[:, k * G3S + m * 128 : k * G3S + (m + 1) * 128],
                                xt_t[:, k * NCHUNK : (k + 1) * NCHUNK],
                                start=(k == 0),
                                stop=(k == KT - 1),
                            )
                        nc.vector.tensor_scalar_add(
                            ev[:, mi * NCHUNK : (mi + 1) * NCHUNK],
                            ps[:, :],
                            biasm_sb[:, m : m + 1],
                        )
                    nc.sync.dma_start(
                        out=xgW[:, ds(c, 1), g, :].rearrange("p o f -> p (o f)"),
                        in_=ev[:, :],
                    )

        # phase 2: GRU scan
        with tc.tile_pool(name="sc_xg", bufs=2) as xg_pool, \
             tc.tile_pool(name="sc_ps", bufs=4, space="PSUM") as sps_pool, \
             tc.tile_pool(name="sc_ew", bufs=6) as ew_pool:
            with tc.For_i(0, NBLK, 1, hint_engines=(mybir.EngineType.PE,)) as blk:
                xgb = xg_pool.tile([128, MT * TB * BC], BF16)
                QBK = NCHUNK // (TB * BC)  # 1 chunk = 8 scan blocks
                for gg in range(NMG):
                    nc.sync.dma_start(
                        out=xgb[:, gg * MG * tb * BC : (gg + 1) * MG * tb * BC]
                        .rearrange("p (m f) -> p m f", m=MG),
                        in_=xgS[
                            :, ds(blk // QBK, 1), ds(gg, 1), :, ds(blk % QBK, 1), :
                        ].rearrange("p c g m q f -> p (c g m) (q f)"),
                    )
                xgb3 = xgb[:, :].rearrange("p (m f) -> p m f", m=MT)
                for tp in range(tb):
                    h_cur = h_a if tp % 2 == 0 else h_b
                    h_nxt = h_b if tp % 2 == 0 else h_a
                    ps_all = sps_pool.tile([128, 192], F32, tag="ps_all")
                    ps_rz = ps_all[:, 0:128]
                    ps_n = ps_all[:, 128:192]
                    for g in (0, 2, 1):
                        for j in range(KT):
                            m = g * KT + j
                            out_ap = (
                                ps_rz[:, g * 64 + j * BC : g * 64 + (j + 1) * BC]
                                if g < 2
                                else ps_n[:, j * BC : (j + 1) * BC]
                            )
                            for k in range(KT):
                                nc.tensor.matmul(
                                    out_ap,
                                    whh_sb[:, k * G3S + m * 128 : k * G3S + (m + 1) * 128],
                                    h_cur[:, k * BC : (k + 1) * BC],
                                    start=(k == 0),
                                    stop=(k == KT - 1),
                                )
                    xg_r = xgb3[:, 0:8, tp * BC : (tp + 1) * BC]
                    xg_z = xgb3[:, 8:16, tp * BC : (tp + 1) * BC]
                    xg_n = xgb3[:, 16:24, tp * BC : (tp + 1) * BC]
                    # r chain (ready after r matmuls; overlaps n matmuls)
                    r_pre = ew_pool.tile([128, JB], F32, tag="r_pre")
                    nc.vector.tensor_tensor(
                        r_pre[:, :].rearrange("p (m f) -> p m f", m=KT),
                        ps_rz[:, 0:64].rearrange("p (m f) -> p m f", m=KT),
                        xg_r,
                        OP.add,
                    )
                    sig_r = ew_pool.tile([128, JB], BF16, tag="sig_r")
                    nc.scalar.activation(sig_r[:, :], r_pre[:, :], AF.Sigmoid)
                    # n chain (ready after n matmuls; overlaps z matmuls)
                    hn = ew_pool.tile([128, JB], F32, tag="hn")
                    nc.vector.tensor_tensor(hn[:, :], ps_n[:, :], bhhn_sb[:, :], OP.add)
                    rhn = ew_pool.tile([128, JB], F32, tag="rhn")
                    nc.vector.tensor_tensor(rhn[:, :], hn[:, :], sig_r[:, :], OP.mult)
                    n_pre = ew_pool.tile([128, JB], F32, tag="n_pre")
                    nc.vector.tensor_tensor(
                        n_pre[:, :].rearrange("p (m f) -> p m f", m=KT),
                        rhn[:, :].rearrange("p (m f) -> p m f", m=KT),
                        xg_n,
                        OP.add,
                    )
                    n_t = ew_pool.tile([128, JB], BF16, tag="n_t")
                    nc.scalar.activation(n_t[:, :], n_pre[:, :], AF.Tanh)
                    d_t = ew_pool.tile([128, JB], BF16, tag="d_t")
                    nc.vector.tensor_tensor(d_t[:, :], h_cur[:, :], n_t[:, :], OP.subtract)
                    # z chain (the only post-z-matmul critical path)
                    z_pre = ew_pool.tile([128, JB], F32, tag="z_pre")
                    nc.vector.tensor_tensor(
                        z_pre[:, :].rearrange("p (m f) -> p m f", m=KT),
                        ps_rz[:, 64:128].rearrange("p (m f) -> p m f", m=KT),
                        xg_z,
                        OP.add,
                    )
                    sig_z = ew_pool.tile([128, JB], BF16, tag="sig_z")
                    nc.scalar.activation(sig_z[:, :], z_pre[:, :], AF.Sigmoid)
                    zd = ew_pool.tile([128, JB], BF16, tag="zd")
                    nc.vector.tensor_tensor(zd[:, :], d_t[:, :], sig_z[:, :], OP.mult)
                    nc.vector.tensor_tensor(h_nxt[:, :], zd[:, :], n_t[:, :], OP.add)

        hT_sb = nc.alloc_sbuf_tensor("hT_sb", [128, JB], F32)
        nc.vector.tensor_copy(hT_sb[:, :], h_a[:, :])
        nc.sync.dma_start(out=hT_out[:, :], in_=hT_sb[:, :])

    nc.compile()
    return nc


def _prep_inputs(inputs):
    batch = np.asarray(inputs["batch"], np.float32)
    W_ih = np.asarray(inputs["W_ih"], np.float32)
    W_hh = np.asarray(inputs["W_hh"], np.float32)
    b_ih = np.asarray(inputs["b_ih"], np.float32)
    b_hh = np.asarray(inputs["b_hh"], np.float32)

    wihT = np.ascontiguousarray(W_ih.T).astype(ml_dtypes.bfloat16)
    whhT = np.ascontiguousarray(W_hh.T).astype(ml_dtypes.bfloat16)
    bias = b_ih.copy()
    bias[: 2 * S] += b_hh[: 2 * S]
    biasm = np.ascontiguousarray(bias.reshape(MT, 128).T).astype(np.float32)
    bn = b_hh[2 * S :].reshape(KT, 128).T
    bhhn = np.repeat(bn[:, :, None], BC, axis=2).reshape(128, JB).astype(np.float32)

    in_maps = []
    for c in range(N_CORES):
        bs = batch[c * BC : (c + 1) * BC]
        xTc = bs.transpose(2, 1, 0).reshape(S, T * BC)
        # chunk-major layout: [chunk, partition, k-tile * in-chunk-token]
        xTc = np.ascontiguousarray(
            xTc.reshape(KT, 128, NXP, NCHUNK)
            .transpose(2, 1, 0, 3)
            .reshape(NXP, 128, KT * NCHUNK)
        )
        in_maps.append({
            "xT": xTc.astype(ml_dtypes.bfloat16),
            "wih": wihT,
            "whh": whhT,
            "biasm": biasm,
            "bhhn": bhhn,
        })
    return in_maps


class _Runner:
    """Compile once, keep the PJRT executable; run per-core in_maps SPMD."""

    def __init__(self, nc, n_cores):
        bass2jax.install_neuronx_cc_hook()
        self.nc, self.n_cores = nc, n_cores
        pname = nc.partition_id_tensor.name if nc.partition_id_tensor else None
        in_names, out_names, out_avals = [], [], []
        for alloc in nc.m.functions[0].allocations:
            if not isinstance(alloc, mybir.MemoryLocationSet):
                continue
            name = alloc.memorylocations[0].name
            if alloc.kind == "ExternalInput":
                if name != pname:
                    in_names.append(name)
            elif alloc.kind == "ExternalOutput":
                out_names.append(name)
                out_avals.append(
                    jax.core.ShapedArray(tuple(alloc.tensor_shape), mybir.dt.np(alloc.dtype))
                )
        self.in_names, self.out_names, self.out_avals = in_names, out_names, out_avals
        n_params = len(in_names)
        all_in = list(in_names) + list(out_names)
        if pname is not None:
            all_in.append(pname)

        def _body(*args):
            operands = list(args)
            if pname is not None:
                operands.append(bass2jax.partition_id_tensor())
            return tuple(
                bass2jax._bass_exec_p.bind(
                    *operands,
                    out_avals=tuple(out_avals),
                    in_names=tuple(all_in),
                    out_names=tuple(out_names),
                    lowering_input_output_aliases=(),
                    sim_require_finite=False,
                    sim_require_nnan=False,
                    nc=nc,
                )
            )

        devices = jax.devices()[:n_cores]
        self.mesh = Mesh(np.asarray(devices), ("core",))
        specs = (PartitionSpec("core"),) * (n_params + len(out_names))
        self.fn = jax.jit(
            shard_map(_body, mesh=self.mesh, in_specs=specs,
                      out_specs=(PartitionSpec("core"),) * len(out_names),
                      check_rep=False),
            keep_unused=True,
        )

    def concat_inputs(self, in_maps):
        concat = [
            np.concatenate([np.asarray(in_maps[c][n]) for c in range(self.n_cores)], axis=0)
            for n in self.in_names
        ]
        zeros = [
            np.zeros((self.n_cores * a.shape[0], *a.shape[1:]), a.dtype)
            for a in self.out_avals
        ]
        return concat + zeros

    def put(self, arrays):
        sh = NamedSharding(self.mesh, PartitionSpec("core"))
        return [jax.device_put(a, sh) for a in arrays]

    def run(self, in_maps):
        dev = self.put(self.concat_inputs(in_maps))
        outs = self.fn(*dev)
        jax.block_until_ready(outs)
        return [
            {
                n: np.asarray(outs[i]).reshape(self.n_cores, *self.out_avals[i].shape)[c]
                for i, n in enumerate(self.out_names)
            }
            for c in range(self.n_cores)
        ]


_CACHED = {}


def kernel(**inputs) -> np.ndarray:
    if "runner" not in _CACHED:
        _CACHED["nc"] = _build()
        _CACHED["runner"] = _Runner(_CACHED["nc"], N_CORES)
    runner = _CACHED["runner"]
    in_maps = _prep_inputs(inputs)
    results = runner.run(in_maps)

    W_out = np.asarray(inputs["W_out"], np.float32)
    b_out = np.asarray(inputs["b_out"], np.float32)
    outs = []
    for c in range(N_CORES):
        hT = np.asarray(results[c]["hT"], np.float32)
        h = hT.reshape(128, KT, BC).transpose(2, 1, 0).reshape(BC, S)
        logits = h @ W_out.T + b_out
        outs.append(1.0 / (1.0 + np.exp(-logits[:, 0])))
    return np.concatenate(outs, 0).astype(np.float32)

